# revision 2
# baseline (speedup 1.0000x reference)
"""Multi-head causal attention with RoPE on 8 TRN2 NeuronCores.

Problem: B=2, T=2048, D=1024, H=16 heads (dh=64), fp32 I/O.
  q/k/v = x @ w{q,k,v}.T ; RoPE(q,k) ; causal softmax((q k^T)/sqrt(dh)) @ v ;
  out = concat_heads @ wo.T

Sharding (8 cores): head-parallel compute, 64-token-sliced output. Core c owns
heads {2c, 2c+1} for both batches; per attention tile t=(b,j) an 8-rank
AllToAll redistributes that tile's normalized attention outputs so core c
ends up with all 1024 features for tokens [512j+64c, 512j+64c+64) of batch b.
Output projection runs per tile-PAIR (64+64 tokens = full 128-row matmuls).

v2 scheduling notes (on top of the v1 lessons kept below):
 - v1 used 4 coarse AllToAlls; the trace showed ~66us of PE idle concentrated
   where finals waited on collective results, plus a 33us tail
   (last epilogue -> cc3 -> at -> final). v2 dispatches one small (128KB)
   AllToAll per tile, so 7/8 of the comm lands while attention still runs and
   the tail carries only one tile.
 - Fillers (projection/final matmuls) are now pulled INSIDE attn_core, one
   after each QK and each PV chunk: the PE stays continuously busy while the
   scalar-engine exp stream paces the attention pipeline (matmul mean was
   425ns = mid p-state; sustained PE occupancy is what lets the clock ramp).
 - at loads (a2a_out -> SBUF) are delayed TWO tiles after their collective's
   dispatch: cc(0..2) queue behind the ~46us CC bootstrap, and a sync-queue
   DMA stalled on a collective semaphore would block the staging DMAs behind
   it (the v1 cascade failure mode).

v1 lessons that still apply:
 - All heavy inputs host-packed so each DMA is one descriptor with 2KB+
   contiguous bytes per partition.
 - PV trails QK by two chunks (pt bufs=4) so scalar-engine exp latency never
   blocks the in-order PE queue.
 - Dummy 8-rank AllToAll dispatched first: CC bootstrap (~46us, serial on the
   CC queue) runs concurrently with the projection phase.
 - NO gpsimd custom-ISA ops (each Pool microcode library swap ~6us dead time).
 - Engine load map: Scalar = exps + vtt copies + prefetch DMAs; DVE = rope
   shuffle/cos-mul/add, o65+fo psum copies, v-transpose copies, onr muls,
   fast reciprocal; Pool = rope sin-mul, causal mask muls, collective
   dispatches (all plain tensor ops, one shared library).
"""

import numpy as np
import ml_dtypes

import concourse.bacc as bacc
import concourse.tile as tile
import concourse.mybir as mybir
from concourse import bass_utils

BF16 = mybir.dt.bfloat16
F32 = mybir.dt.float32
AF = mybir.ActivationFunctionType

NCORES = 8
B, T, D, H = 2, 2048, 1024, 16
DH = D // H          # 64
HPC = H // NCORES    # 2 heads per core
FPC = DH * HPC       # 128 features per core
TOK = B * T          # 4096
TPC = TOK // NCORES  # 512 tokens per core (output shard)
KC = D // 128        # 8 contraction chunks
NT = T // 512        # 4 query tiles of 512 per batch
NTILE = B * NT       # 8 attention tiles
VG = 256             # cols per v-group: [v_h0(64) | 1 | pad | v_h1(64) | 1 | pad]

_COMPILED = None


def _build():
    nc = bacc.Bacc("TRN2", target_bir_lowering=False, debug=False, num_devices=NCORES)

    xp_d = nc.dram_tensor("xp", [128, KC * TOK], BF16, kind="ExternalInput")
    wq_d = nc.dram_tensor("wqp", [128, KC * FPC], BF16, kind="ExternalInput")
    wk_d = nc.dram_tensor("wkp", [128, KC * FPC], BF16, kind="ExternalInput")
    wv_d = nc.dram_tensor("wvp", [128, KC * FPC], BF16, kind="ExternalInput")
    wo_d = nc.dram_tensor("wop", [128, KC * D], BF16, kind="ExternalInput")
    C_d = nc.dram_tensor("cosC", [128, T], BF16, kind="ExternalInput")
    S_d = nc.dram_tensor("sinS", [128, T], BF16, kind="ExternalInput")
    mask_d = nc.dram_tensor("mask", [128, 128], BF16, kind="ExternalInput")
    id_d = nc.dram_tensor("ident", [128, 128], BF16, kind="ExternalInput")
    sel_d = nc.dram_tensor("sel2", [2, 128], BF16, kind="ExternalInput")
    out_d = nc.dram_tensor("out", [TPC, D], F32, kind="ExternalOutput")

    swap16 = list(range(16, 32)) + list(range(16))

    with tile.TileContext(nc) as tc:
        with (
            tc.tile_pool(name="sb", bufs=1) as sb,
            tc.tile_pool(name="ps", bufs=1, space="PSUM") as ps,
            tc.tile_pool(name="dram", bufs=1, space="DRAM") as dram,
        ):
            # ---- dummy collective first: pays the CC bootstrap cost during
            # the projection phase ----
            dum_in = dram.tile([8, 16], BF16, name="dumin")
            dum_out = dram.tile([8, 16], BF16, name="dumout")
            zz = sb.tile([8, 16], BF16)
            nc.gpsimd.memset(zz[:], 0.0)
            nc.gpsimd.dma_start(dum_in[:], zz[:])
            nc.gpsimd.collective_compute(
                "AllToAll",
                mybir.AluOpType.bypass,
                replica_groups=[list(range(NCORES))],
                ins=[dum_in.opt()],
                outs=[dum_out.opt()],
            )

            # ---- prefetch: everything is host-packed, one flat DMA each.
            wq_sb = sb.tile([128, KC * FPC], BF16)
            wk_sb = sb.tile([128, KC * FPC], BF16)
            wv_sb = sb.tile([128, KC * FPC], BF16)
            C_sb = sb.tile([128, T], BF16)
            S_sb = sb.tile([128, T], BF16)
            mask2_sb = sb.tile([128, 256], BF16)
            id_sb = sb.tile([128, 128], BF16)
            xp_sb = sb.tile([128, KC * TOK], BF16)
            wo_sb = sb.tile([128, KC * D], BF16)

            BLK = KC * 512  # 4096 cols per (b,n) token block

            def x_block(i):
                return (
                    xp_sb[:, BLK * i : BLK * i + BLK],
                    xp_d[:, BLK * i : BLK * i + BLK],
                )

            nc.scalar.dma_start(wq_sb[:], wq_d[:])
            d, s = x_block(0)
            half = KC * 256
            nc.sync.dma_start(d[:, 0:half], s[:, 0:half])
            nc.scalar.dma_start(d[:, half:BLK], s[:, half:BLK])
            nc.scalar.dma_start(wk_sb[:], wk_d[:])
            nc.scalar.dma_start(wv_sb[:], wv_d[:])
            nc.sync.dma_start(C_sb[:], C_d[:])
            nc.sync.dma_start(S_sb[:], S_d[:])
            nc.sync.dma_start(mask2_sb[:, 0:128], mask_d[:])
            nc.sync.dma_start(mask2_sb[:, 128:256], mask_d[:])
            nc.sync.dma_start(id_sb[:], id_d[:])
            for i in range(1, B * NT):
                d, s = x_block(i)
                (nc.scalar if i % 2 else nc.sync).dma_start(d, s)
            nc.scalar.dma_start(wo_sb[:], wo_d[:])

            # ---- persistent intermediates ----
            qrot_sb = sb.tile([128, TOK], BF16)
            krot_sb = sb.tile([128, TOK], BF16)
            v1_sb = sb.tile([128, B * (T // 128) * VG], BF16)
            nc.gpsimd.memset(
                v1_sb[:].rearrange("p (g c) -> p g c", c=128)[:, :, 64:65], 1.0
            )

            # per-tile AllToAll buffers: tile t sends [1024, 64] (8 blocks of
            # [128 feat, 64 toks]; block o = my 2 heads' features for the 64
            # tokens core o owns in tile t). Receives block r = core r's 128
            # features for MY 64 tokens.
            a2a_in = [dram.tile([D, 64], BF16, name=f"a2ain{t}") for t in range(NTILE)]
            a2a_out = [dram.tile([D, 64], BF16, name=f"a2aout{t}") for t in range(NTILE)]

            # ================= filler machinery =================
            fillers = []  # FIFO of zero-arg thunks, each ~1 PE instruction

            def pull(k):
                for _ in range(k):
                    if fillers:
                        fillers.pop(0)()

            def drain():
                while fillers:
                    fillers.pop(0)()

            def rope_tile(pp, dst_sb, b, n):
                swp = sb.tile([128, 512], F32, tag="swp", bufs=3, name=f"swp{b}{n}")
                nc.vector.stream_shuffle(swp[:], pp[:], swap16)
                t1 = sb.tile([128, 512], BF16, tag="t1", bufs=3, name=f"t1{b}{n}")
                nc.vector.tensor_mul(t1[:], pp[:], C_sb[:, 512 * n : 512 * n + 512])
                t2 = sb.tile([128, 512], BF16, tag="t2", bufs=3, name=f"t2{b}{n}")
                nc.gpsimd.tensor_mul(t2[:], swp[:], S_sb[:, 512 * n : 512 * n + 512])
                nc.vector.tensor_add(
                    dst_sb[:, b * T + 512 * n : b * T + 512 * n + 512], t1[:], t2[:]
                )

            def add_proj_fillers(w_sb, b, n, kind):
                st = {}
                blk = (NT * b + n) * BLK

                def mk(kc):
                    def f():
                        if kc == 0:
                            st["pp"] = ps.tile(
                                [128, 512], F32, tag="proj", bufs=2, name=f"pp{kind}{b}{n}"
                            )
                        nc.tensor.matmul(
                            st["pp"][:],
                            w_sb[:, kc * FPC : (kc + 1) * FPC],
                            xp_sb[:, blk + 512 * kc : blk + 512 * kc + 512],
                            start=(kc == 0),
                            stop=(kc == KC - 1),
                        )
                        if kc == KC - 1:
                            if kind == "q":
                                rope_tile(st["pp"], qrot_sb, b, n)
                            elif kind == "k":
                                rope_tile(st["pp"], krot_sb, b, n)
                            else:
                                vtt = sb.tile(
                                    [128, 512], BF16, tag="vtt", bufs=2, name=f"vtt{b}{n}"
                                )
                                nc.scalar.activation(vtt[:], st["pp"][:], AF.Copy)
                                for i in range(4):
                                    fillers.append(mk_transpose(vtt, b, n, i))

                    return f

                for kc in range(KC):
                    fillers.append(mk(kc))

            def mk_transpose(vtt, b, n, i):
                def f():
                    g = VG * ((T // 128) * b + 4 * n + i)
                    tp = ps.tile([128, 128], BF16, tag="proj", bufs=2, name=f"tp{b}{n}{i}")
                    nc.tensor.matmul(
                        tp[:],
                        vtt[:, 128 * i : 128 * i + 128],
                        id_sb[:],
                        is_transpose=True,
                        start=True,
                        stop=True,
                    )
                    nc.vector.tensor_copy(
                        v1_sb[:, g : g + 256].rearrange("p (h c) -> p h c", h=2)[
                            :, :, 0:64
                        ],
                        tp[:].rearrange("p (h c) -> p h c", h=2),
                    )

                return f

            def add_tile(t):
                b, n = t // NT, t % NT
                add_proj_fillers(wq_sb, b, n, "q")
                add_proj_fillers(wk_sb, b, n, "k")
                add_proj_fillers(wv_sb, b, n, "v")

            # ---- at tiles (a2a_out -> SBUF): one strided DMA per tile into
            # the tile-pair's at buffer half ----
            at_tiles = {}

            def at_tile_for(p):
                if p not in at_tiles:
                    at_tiles[p] = sb.tile(
                        [128, KC * 128], BF16, tag="at", bufs=2, name=f"at{p}"
                    )
                return at_tiles[p]

            def load_at(t, queues=None):
                p, hh = t // 2, t % 2
                at = at_tile_for(p)
                dst = at[:].rearrange("p (k t2) -> p k t2", t2=128)[
                    :, :, 64 * hh : 64 * hh + 64
                ]
                src = a2a_out[t][:].rearrange("(k p) t2 -> p k t2", p=128)
                if queues is None:
                    nc.sync.dma_start(dst, src)
                else:
                    # tail: split feature chunks across queues
                    nq = len(queues)
                    per = (KC + nq - 1) // nq
                    for qi, q in enumerate(queues):
                        k0, k1 = qi * per, min(KC, (qi + 1) * per)
                        if k0 < k1:
                            q.dma_start(dst[:, k0:k1, :], src[:, k0:k1, :])

            def add_final_fillers(p):
                """16 fillers: output projection for tile pair p (tiles 2p,
                2p+1; out rows 0-63 = tile 2p's 64 tokens, 64-127 = 2p+1's)."""
                st = {}

                def mk(nh, kc):
                    def f():
                        at = at_tiles[p]
                        if kc == 0:
                            st[nh] = ps.tile(
                                [128, 512], F32, tag="proj", bufs=2, name=f"fp{p}{nh}"
                            )
                        nc.tensor.matmul(
                            st[nh][:],
                            at[:, 128 * kc : 128 * kc + 128],
                            wo_sb[:, kc * D + 512 * nh : kc * D + 512 * nh + 512],
                            start=(kc == 0),
                            stop=(kc == KC - 1),
                        )
                        if kc == KC - 1:
                            fo = sb.tile(
                                [128, 512], F32, tag="fo", bufs=2, name=f"fo{p}{nh}"
                            )
                            nc.vector.tensor_copy(fo[:], st[nh][:])
                            nc.sync.dma_start(
                                out_d[128 * p : 128 * p + 128, 512 * nh : 512 * nh + 512],
                                fo[:],
                            )

                    return f

                for nh in range(2):
                    for kc in range(KC):
                        fillers.append(mk(nh, kc))

            # sel2: [2,128] selection matrix for the PE-side denominator
            # broadcast (row h -> output partitions 64h..64h+64)
            sel2 = sb.tile([2, 128], BF16)
            nc.sync.dma_start(sel2[:], sel_d[:])

            # ================= attention =================
            def attn_core(b, j):
                """Both heads for (batch b, q-tile j). One filler is pulled
                after every QK and every PV chunk so the PE never idles while
                the scalar-engine exp stream paces the pipeline."""
                ops = [
                    ps.tile([65, 512], F32, tag="opsum", bufs=2, name=f"op{b}{h}{j}")
                    for h in range(2)
                ]
                nch = 4 * j + 4

                def qk_exp(c):
                    diag = c - 4 * j
                    lo = 128 * diag if diag >= 0 else 0
                    sp = ps.tile(
                        [128, 1024], F32, tag="spsum", bufs=2, name=f"sp{b}{j}{c}"
                    )
                    spv = sp[:].rearrange("p (h t) -> p h t", h=2)
                    for h in range(2):
                        nc.tensor.matmul(
                            sp[:, 512 * h + lo : 512 * h + 512],
                            krot_sb[64 * h : 64 * h + 64, b * T + 128 * c : b * T + 128 * c + 128],
                            qrot_sb[
                                64 * h : 64 * h + 64,
                                b * T + 512 * j + lo : b * T + 512 * j + 512,
                            ],
                            start=True,
                            stop=True,
                        )
                    pt = sb.tile(
                        [128, 1024], BF16, tag="pt", bufs=4, name=f"pt{b}{j}{c}"
                    )
                    ptv = pt[:].rearrange("p (h t) -> p h t", h=2)
                    nc.scalar.activation(
                        ptv[:, :, lo:512], spv[:, :, lo:512], AF.Exp, scale=0.125
                    )
                    if diag >= 0:
                        nc.gpsimd.tensor_mul(
                            ptv[:, :, lo : lo + 128], ptv[:, :, lo : lo + 128],
                            mask2_sb[:].rearrange("p (h t) -> p h t", h=2),
                        )
                    return pt

                def pv(c, pt):
                    diag = c - 4 * j
                    lo = 128 * diag if diag >= 0 else 0
                    g = VG * ((T // 128) * b + c)
                    for h in range(2):
                        nc.tensor.matmul(
                            ops[h][:, lo:512],
                            v1_sb[:, g + 128 * h : g + 128 * h + 65],
                            pt[:, 512 * h + lo : 512 * h + 512],
                            start=(c == 0),
                            stop=(c == nch - 1),
                        )

                pts = {}
                for c in range(nch):
                    pts[c] = qk_exp(c)
                    pull(1)
                    if c >= 2:
                        pv(c - 2, pts.pop(c - 2))
                        pull(1)
                pv(nch - 2, pts.pop(nch - 2))
                pull(1)
                pv(nch - 1, pts.pop(nch - 1))
                pull(1)
                o65s = []
                for h in range(2):
                    o65 = sb.tile([65, 512], F32, tag="o65", bufs=4, name=f"o65{b}{h}{j}")
                    nc.vector.tensor_copy(o65[:], ops[h][:])
                    o65s.append(o65)
                return o65s

            def epilogue_a(t, o65s):
                """DVE + sync only. Returns part_b closure which also stages
                and dispatches this tile's AllToAll."""
                b, j = t // NT, t % NT
                sums = sb.tile([2, 512], F32, tag="sums", bufs=3, name=f"sums{b}{j}")
                for h in range(2):
                    nc.sync.dma_start(sums[h : h + 1, :], o65s[h][64:65, :])
                rec2 = sb.tile([2, 512], F32, tag="rec4", bufs=3, name=f"rec2{b}{j}")
                nc.vector.reciprocal_approx_fast(rec2[:], sums[:])
                recb2 = sb.tile([2, 512], BF16, tag="recb2", bufs=3, name=f"recb2{b}{j}")
                nc.vector.tensor_copy(recb2[:], rec2[:])

                def part_b():
                    bps = ps.tile([128, 512], F32, tag="spsum", bufs=2, name=f"bps{b}{j}")
                    nc.tensor.matmul(bps[:], sel2[:], recb2[:], start=True, stop=True)
                    for h in range(2):
                        onr = sb.tile([64, 512], BF16, tag="onr", bufs=4, name=f"onr{b}{j}{h}")
                        nc.vector.tensor_mul(
                            onr[:], o65s[h][0:64, :], bps[64 * h : 64 * h + 64, :]
                        )
                        # block o = my features for the 64 tokens core o owns
                        nc.sync.dma_start(
                            a2a_in[t][:]
                            .rearrange("(o r) t2 -> r o t2", r=128)[
                                64 * h : 64 * h + 64, :, :
                            ],
                            onr[:].rearrange("p (o t2) -> p o t2", o=NCORES),
                        )
                    nc.gpsimd.collective_compute(
                        "AllToAll",
                        mybir.AluOpType.bypass,
                        replica_groups=[list(range(NCORES))],
                        ins=[a2a_in[t].opt()],
                        outs=[a2a_out[t].opt()],
                    )

                return part_b

            # ================= schedule =================
            add_tile(0)
            drain()
            add_tile(1)
            drain()
            add_tile(2)  # filler supply for attn(0)

            def block(pb, adds=(), at_loads=(), finals=()):
                for t in adds:
                    add_tile(t)
                pull(8)
                pb()
                for t in at_loads:
                    load_at(t)
                for p in finals:
                    add_final_fillers(p)
                drain()

            pb = epilogue_a(0, attn_core(0, 0))
            block(pb, adds=(3,))
            pb = epilogue_a(1, attn_core(0, 1))
            block(pb, adds=(4,))
            pb = epilogue_a(2, attn_core(0, 2))
            block(pb, adds=(5,))
            pb = epilogue_a(3, attn_core(0, 3))
            block(pb, adds=(6,), at_loads=(0, 1), finals=(0,))
            pb = epilogue_a(4, attn_core(1, 0))
            block(pb, adds=(7,), at_loads=(2, 3), finals=(1,))
            pb = epilogue_a(5, attn_core(1, 1))
            block(pb)
            pb = epilogue_a(6, attn_core(1, 2))
            block(pb, at_loads=(4, 5), finals=(2,))
            pb = epilogue_a(7, attn_core(1, 3))
            block(pb, at_loads=(6,))
            load_at(7, queues=[nc.sync, nc.scalar, nc.gpsimd])
            add_final_fillers(3)
            drain()

    nc.compile()
    return nc


def _get_compiled():
    global _COMPILED
    if _COMPILED is None:
        _COMPILED = _build()
    return _COMPILED


def _prep_in_maps(embedding_word, wq, wk, wv, wo):
    bf = ml_dtypes.bfloat16
    x = np.asarray(embedding_word, np.float32).reshape(TOK, D)
    xT = np.ascontiguousarray(x.T).astype(bf)  # [D, TOK]
    xp = np.ascontiguousarray(
        xT.reshape(KC, 128, B, NT, 512).transpose(1, 2, 3, 0, 4).reshape(128, KC * TOK)
    )

    woT = np.asarray(wo, np.float32).T  # [D, D]
    wop = np.ascontiguousarray(
        woT.reshape(KC, 128, D).transpose(1, 0, 2).reshape(128, KC * D)
    ).astype(bf)

    perm64 = [
        (2 * (16 * q + r) if r < 16 else 2 * (16 * q + (r - 16)) + 1)
        for q in range(2)
        for r in range(32)
    ]
    perm64 = np.asarray(perm64)

    freqs = 1.0 / (10000.0 ** (np.arange(0, DH, 2, dtype=np.float64) / DH))  # [32]
    ang = np.arange(T, dtype=np.float64)[:, None] * freqs[None, :]  # [T, 32]
    cos_t, sin_t = np.cos(ang), np.sin(ang)
    rows = np.arange(128)
    wh = rows % 64
    qd = wh // 32
    r32 = wh % 32
    dmap = 16 * qd + (r32 % 16)
    sign = np.where(r32 < 16, -1.0, 1.0)
    C = np.ascontiguousarray(cos_t[:, dmap].T).astype(bf)  # [128, T]
    S = np.ascontiguousarray((sin_t[:, dmap] * sign[None, :]).T).astype(bf)

    rr = np.arange(128)[:, None]
    cc = np.arange(128)[None, :]
    mask = np.where(cc >= rr, 1.0, 0.0).astype(bf)
    ident = np.eye(128, dtype=np.float32).astype(bf)
    sel2 = np.zeros((2, 128), np.float32)
    sel2[0, 0:64] = 1.0
    sel2[1, 64:128] = 1.0
    sel2 = sel2.astype(bf)

    wqf = np.asarray(wq, np.float32)
    wkf = np.asarray(wk, np.float32)
    wvf = np.asarray(wv, np.float32)

    def pack_w(w_c):
        wT = w_c.T
        return np.ascontiguousarray(
            wT.reshape(KC, 128, FPC).transpose(1, 0, 2).reshape(128, KC * FPC)
        ).astype(bf)

    in_maps = []
    for c in range(NCORES):
        rows_c = slice(FPC * c, FPC * c + FPC)
        wq_c = wqf[rows_c].reshape(HPC, DH, D)[:, perm64, :].reshape(FPC, D)
        wk_c = wkf[rows_c].reshape(HPC, DH, D)[:, perm64, :].reshape(FPC, D)
        wv_c = wvf[rows_c]
        in_maps.append(
            {
                "xp": xp,
                "wqp": pack_w(wq_c),
                "wkp": pack_w(wk_c),
                "wvp": pack_w(wv_c),
                "wop": wop,
                "cosC": C,
                "sinS": S,
                "mask": mask,
                "ident": ident,
                "sel2": sel2,
            }
        )
    return in_maps


def _unshard(core_outs):
    """core_outs[c] is [TPC, D]: row 64*t+i = token (b=t//4, 512*(t%4)+64*c+i).
    Interleave back to [B, T, D]."""
    a = np.stack(core_outs, axis=0)  # [8, 512, D]
    a = a.reshape(NCORES, NTILE, 64, D).transpose(1, 0, 2, 3).reshape(TOK, D)
    return np.ascontiguousarray(a.reshape(B, T, D).astype(np.float32))


def kernel(embedding_word, wq, wk, wv, wo):
    nc = _get_compiled()
    in_maps = _prep_in_maps(embedding_word, wq, wk, wv, wo)
    res = bass_utils.run_bass_kernel_spmd(nc, in_maps, core_ids=list(range(NCORES)))
    return _unshard([res.results[c]["out"] for c in range(NCORES)])
